# revision 2
# baseline (speedup 1.0000x reference)
import sys

sys.path.insert(0, "/opt/trn_rl_repo")

import numpy as np

import concourse.bass as bass
import concourse.mybir as mybir
import concourse.tile as tile
from concourse.bacc import Bacc
from concourse.bass_utils import run_bass_kernel_spmd

B, C, H, W = 2, 3, 160, 160
L, NCB = 72, 3000
S, KK = 4, 5
PAD = KK // 2
NCORES = 8
HB = H // 4  # 40 h-rows per core
NT = 50  # tiles per core: 10 row-groups x 5 col-groups
TP = 128  # pixels per tile: 4 rows x 32 cols
CHUNKS = [512, 512, 512, 512, 512, 440]


def _build_nc():
    nc = Bacc()
    qs_d = nc.dram_tensor("qs", [L, NT, TP], mybir.dt.float32, kind="ExternalInput")
    keysT_d = nc.dram_tensor("keysT", [L, NCB], mybir.dt.float32, kind="ExternalInput")
    vals_d = nc.dram_tensor(
        "vals", [NCB, S * S * KK * KK], mybir.dt.float32, kind="ExternalInput"
    )
    patches_d = nc.dram_tensor(
        "patches", [NT, TP, C, KK * KK], mybir.dt.float32, kind="ExternalInput"
    )
    out_d = nc.dram_tensor(
        "out_raw", [NT, TP, C * S * S], mybir.dt.float32, kind="ExternalOutput"
    )

    with tile.TileContext(nc) as tc:
        with (
            tc.tile_pool(name="persist", bufs=1) as pp,
            tc.tile_pool(name="sim", bufs=2) as simp,
            tc.tile_pool(name="work", bufs=2) as wp,
            tc.tile_pool(name="ps", bufs=8, space="PSUM") as ps,
        ):
            keysT_t = pp.tile([L, NCB], mybir.dt.float32)
            qs_t = pp.tile([L, NT, TP], mybir.dt.float32)
            nc.sync.dma_start(keysT_t[:], keysT_d[:])
            nc.sync.dma_start(qs_t[:], qs_d[:])

            for t in range(NT):
                patches_t = wp.tile([TP, C, KK * KK], mybir.dt.float32)
                nc.sync.dma_start(patches_t[:], patches_d[t])

                sim_sb = simp.tile([TP, NCB], mybir.dt.float32)
                c0 = 0
                for cw in CHUNKS:
                    pj = ps.tile([TP, 512], mybir.dt.float32)
                    nc.tensor.matmul(
                        out=pj[:, :cw],
                        lhsT=qs_t[:, t, :],
                        rhs=keysT_t[:, c0 : c0 + cw],
                        start=True,
                        stop=True,
                    )
                    nc.scalar.copy(sim_sb[:, c0 : c0 + cw], pj[:, :cw])
                    c0 += cw

                max8 = wp.tile([TP, 8], mybir.dt.float32)
                idx8 = wp.tile([TP, 8], mybir.dt.uint32)
                nc.vector.max(max8[:], sim_sb[:])
                nc.vector.max_index(idx8[:], max8[:], sim_sb[:])
                idx32 = wp.tile([TP, 1], mybir.dt.int32)
                nc.vector.tensor_copy(idx32[:], idx8[:, 0:1])

                v_t = wp.tile([TP, S * S * KK * KK], mybir.dt.float32)
                nc.gpsimd.indirect_dma_start(
                    out=v_t[:],
                    out_offset=None,
                    in_=vals_d[:],
                    in_offset=bass.IndirectOffsetOnAxis(ap=idx32[:, :1], axis=0),
                )
                v3 = v_t[:].rearrange("p (s k) -> p s k", s=S * S)

                prod = wp.tile([TP, C, S * S, KK * KK], mybir.dt.float32)
                nc.vector.tensor_tensor(
                    out=prod[:],
                    in0=patches_t[:].unsqueeze(2).to_broadcast([TP, C, S * S, KK * KK]),
                    in1=v3.unsqueeze(1).to_broadcast([TP, C, S * S, KK * KK]),
                    op=mybir.AluOpType.mult,
                )
                conv = wp.tile([TP, C, S * S], mybir.dt.float32)
                nc.vector.tensor_reduce(
                    out=conv[:],
                    in_=prod[:],
                    axis=mybir.AxisListType.X,
                    op=mybir.AluOpType.add,
                )
                nc.sync.dma_start(out_d[t], conv[:])

    nc.finalize()
    return nc


def _prep_inputs(x, queries, keys, values):
    xp = np.pad(x, ((0, 0), (0, 0), (PAD, PAD), (PAD, PAD)), mode="reflect")
    # win[b, c, h, w, ky, kx] = xp[b, c, h+ky, w+kx]
    win = np.lib.stride_tricks.sliding_window_view(xp, (KK, KK), axis=(2, 3))
    keysT = np.ascontiguousarray(keys.T)
    vals = np.ascontiguousarray(values.reshape(NCB, S * S * KK * KK))
    in_maps = []
    for core in range(NCORES):
        b, h0 = core // 4, (core % 4) * HB
        # queries [L, 40, 160] -> [L, r, dr, cb, dw] -> [L, r, cb, dr, dw] -> [L, 50, 128]
        q = queries[b, :, h0 : h0 + HB, :].reshape(L, 10, 4, 5, 32)
        q = np.ascontiguousarray(q.transpose(0, 1, 3, 2, 4)).reshape(L, NT, TP)
        # patches [c, 40, 160, ky, kx] -> [c, r, dr, cb, dw, ky, kx]
        p = win[b, :, h0 : h0 + HB, :, :, :].reshape(C, 10, 4, 5, 32, KK, KK)
        # -> [r, cb, dr, dw, c, ky, kx] -> [50, 128, 3, 25]
        p = np.ascontiguousarray(p.transpose(1, 3, 2, 4, 0, 5, 6)).reshape(
            NT, TP, C, KK * KK
        )
        in_maps.append({"qs": q, "keysT": keysT, "vals": vals, "patches": p})
    return in_maps


def _assemble(results):
    out = np.empty((B, C, H * S, W * S), dtype=np.float32)
    for core in range(NCORES):
        b, h0 = core // 4, (core % 4) * HB
        raw = results[core]["out_raw"]  # [50, 128, 48]
        # [r, cb, dr, dw, c, sy, sx] -> [c, r, dr, sy, cb, dw, sx]
        r = raw.reshape(10, 5, 4, 32, C, S, S).transpose(4, 0, 2, 5, 1, 3, 6)
        out[b, :, S * h0 : S * (h0 + HB), :] = r.reshape(C, HB * S, W * S)
    return out


def kernel(x, queries, keys, values, s, k):
    assert int(s) == S and int(k) == KK
    x = np.asarray(x, dtype=np.float32)
    queries = np.asarray(queries, dtype=np.float32)
    keys = np.asarray(keys, dtype=np.float32)
    values = np.asarray(values, dtype=np.float32)

    nc = _build_nc()
    in_maps = _prep_inputs(x, queries, keys, values)
    res = run_bass_kernel_spmd(nc, in_maps, list(range(NCORES)))
    return _assemble(res.results)


if __name__ == "__main__":
    rng = np.random.default_rng(0)
    out = kernel(
        x=rng.standard_normal((B, C, H, W), dtype=np.float32),
        queries=rng.standard_normal((B, L, H, W), dtype=np.float32),
        keys=rng.standard_normal((NCB, L), dtype=np.float32),
        values=rng.standard_normal((NCB, S * S, KK * KK), dtype=np.float32),
        s=S,
        k=KK,
    )
    print(out.shape, out.dtype)
